# revision 7
# baseline (speedup 1.0000x reference)
"""Trainium2 Bass kernel for a 4-layer decoder transformer (B4 T1024 E1024 H16
hs64 F4096 V32000) on 8 NeuronCores.

Sharding: batch(4) x sequence-half(2). Core c handles batch b=c//2, tokens
[half*512, half*512+512) with half=c%2. The residual stream lives in SBUF
transposed (xT: [E, 512], E on partitions) so every matmul has its
contraction dim on partitions. The only cross-core exchange is a pair
AllGather of (kT | v) per layer; causality is enforced with per-core binary
mask inputs so the 8-core program is uniform (SPMD).

All model math (embedding gather, 4 transformer layers, final LN, lm_head)
runs on device; the host only slices/reorders/casts inputs and concatenates
the 8 output shards.
"""

import numpy as np
import ml_dtypes

import concourse.bass as bass
import concourse.bacc as bacc
import concourse.mybir as mybir
import concourse.tile as tile
from concourse import bass_utils
from concourse.masks import make_identity

F32 = mybir.dt.float32
BF16 = mybir.dt.bfloat16
I32 = mybir.dt.int32
AF = mybir.ActivationFunctionType
OP = mybir.AluOpType
P = 128

N_CORES = 8
PAIRS = [[0, 1], [2, 3], [4, 5], [6, 7]]


class Cfg:
    def __init__(self, B=4, T=1024, E=1024, H=16, HS=64, L=4, F=4096, V=32000):
        self.B, self.T, self.E, self.H, self.HS = B, T, E, H, HS
        self.L, self.F, self.V = L, F, V
        self.TC = T // 2                    # tokens per core
        self.NEC = E // P                   # E chunks (partition tiles)
        self.NTC = self.TC // P             # local token chunks
        self.NKC = T // P                   # global key chunks
        self.NFC = F // P                   # FFN hidden chunks
        self.HPP = P // HS                  # heads per 128-partition tile
        self.NHP = (H * HS) // P            # head-pair tiles
        self.HP = HS + 1                    # augmented per-head stride in v
        self.scale = 1.0 / (E ** 0.5)
        # lm_head vocab chunks of <=512 cols
        self.vchunks = []
        v0 = 0
        while v0 < V:
            self.vchunks.append((v0, min(512, V - v0)))
            v0 += 512
        # element offsets inside the AG payload (bf16 elems)
        self.k_elems = H * HS * self.TC           # kT shard elems
        self.vrow = H * self.HP                   # augmented v row width
        self.v_elems = self.TC * self.vrow
        self.shard_elems = self.k_elems + self.v_elems


def build_program(c: Cfg):
    nc = bacc.Bacc("TRN2", target_bir_lowering=False, debug=False,
                   num_devices=N_CORES)

    # ---- DRAM I/O ----
    dt_ = nc.dram_tensor
    idx_t = dt_("idx", [c.TC], I32, kind="ExternalInput").ap()
    temb_t = dt_("temb", [c.V, c.E], BF16, kind="ExternalInput").ap()
    posT_t = dt_("posT", [c.E, c.TC], F32, kind="ExternalInput").ap()
    wqkv_t = dt_("wqkv", [c.L, c.E, 3 * c.H * c.HS], BF16,
                 kind="ExternalInput").ap()
    wo_t = dt_("wo", [c.L, c.E, c.E], BF16, kind="ExternalInput").ap()
    bo_t = dt_("bo", [c.L, c.E], F32, kind="ExternalInput").ap()
    ln1g_t = dt_("ln1g", [c.L, c.E], F32, kind="ExternalInput").ap()
    ln1b_t = dt_("ln1b", [c.L, c.E], F32, kind="ExternalInput").ap()
    ln2g_t = dt_("ln2g", [c.L, c.E], F32, kind="ExternalInput").ap()
    ln2b_t = dt_("ln2b", [c.L, c.E], F32, kind="ExternalInput").ap()
    w1_t = dt_("w1", [c.L, c.E, c.F], BF16, kind="ExternalInput").ap()
    b1_t = dt_("b1", [c.L, c.F], F32, kind="ExternalInput").ap()
    w2_t = dt_("w2", [c.L, c.F, c.E], BF16, kind="ExternalInput").ap()
    b2_t = dt_("b2", [c.L, c.E], F32, kind="ExternalInput").ap()
    lnfg_t = dt_("lnfg", [c.E], F32, kind="ExternalInput").ap()
    lnfb_t = dt_("lnfb", [c.E], F32, kind="ExternalInput").ap()
    wh_t = dt_("wh", [c.E, c.V], BF16, kind="ExternalInput").ap()
    bh_t = dt_("bh", [c.V], F32, kind="ExternalInput").ap()
    mask_t = dt_("mask", [c.NKC, P, c.TC], BF16, kind="ExternalInput").ap()
    out_t = dt_("out", [c.TC, c.V], F32, kind="ExternalOutput").ap()

    with tile.TileContext(nc) as tc:
        with tc.tile_pool(name="sb", bufs=1) as sb, \
             tc.tile_pool(name="wpool", bufs=3) as wpool, \
             tc.tile_pool(name="xpool", bufs=2) as xpool, \
             tc.tile_pool(name="ps", bufs=8, space="PSUM") as ps, \
             tc.tile_pool(name="dram", bufs=2, space="DRAM") as dram:

            def psum(shape=None, dtype=F32, name="mm"):
                return ps.tile(shape or [P, 512], dtype, tag="mm", name=name)

            # ---- constants ----
            ones_bf = sb.tile([P, 1], BF16, tag="ones_bf", name="ones_bf")
            nc.vector.memset(ones_bf[:], 1.0)
            ident = sb.tile([P, P], BF16, tag="ident", name="ident")
            make_identity(nc, ident[:])
            mask_sb = sb.tile([P, c.NKC, c.TC], BF16, tag="mask", name="mask_sb")
            for kc in range(c.NKC):
                nc.sync.dma_start(mask_sb[:, kc, :], mask_t[kc])
            idx_sb = sb.tile([P, c.NTC], I32, tag="idx", name="idx_sb")
            nc.sync.dma_start(idx_sb[:], idx_t.rearrange("(tc p) -> p tc", p=P))

            # ---- residual stream xT[e, t] (f32), seeded with pos_emb^T ----
            xT = sb.tile([P, c.NEC, c.TC], F32, tag="xT", name="xT")
            for ec in range(c.NEC):
                nc.sync.dma_start(xT[:, ec, :], posT_t[ec * P:(ec + 1) * P, :])

            # ---- embedding gather + transpose (xT += gather(temb, idx)^T) ----
            for tcb in range(c.NTC):
                emb = xpool.tile([P, c.E], BF16, tag="emb", name="emb")
                nc.gpsimd.indirect_dma_start(
                    out=emb[:], out_offset=None, in_=temb_t,
                    in_offset=bass.IndirectOffsetOnAxis(
                        ap=idx_sb[:, tcb:tcb + 1], axis=0))
                for ec in range(c.NEC):
                    tps = psum([P, P], BF16, name="tps")
                    nc.tensor.transpose(
                        out=tps[:], in_=emb[:, ec * P:(ec + 1) * P],
                        identity=ident[:])
                    tpf = xpool.tile([P, P], F32, tag="tpf", name="tpf")
                    nc.vector.tensor_copy(out=tpf[:], in_=tps[:])
                    sl = xT[:, ec, tcb * P:(tcb + 1) * P]
                    nc.vector.tensor_tensor(out=sl, in0=sl, in1=tpf[:], op=OP.add)

            # ---- layernorm helper: xT -> out_bf (bf16 [P, NEC, TC]) ----
            def layernorm(g_dram, b_dram, out_bf):
                gb = sb.tile([P, 2 * c.NEC], F32, tag="gains", name="gb", bufs=2)
                nc.sync.dma_start(
                    gb[:, 0:c.NEC], g_dram.rearrange("(ec p) -> p ec", p=P))
                nc.sync.dma_start(
                    gb[:, c.NEC:], b_dram.rearrange("(ec p) -> p ec", p=P))
                sum_ps = psum([1, c.TC], name="ln_sum")
                sq_ps = psum([1, c.TC], name="ln_sq")
                for ec in range(c.NEC):
                    xbf = xpool.tile([P, c.TC], BF16, tag="xbf", name="xbf")
                    nc.vector.tensor_copy(out=xbf[:], in_=xT[:, ec, :])
                    nc.tensor.matmul(out=sum_ps[:], lhsT=ones_bf[:], rhs=xbf[:],
                                     start=(ec == 0), stop=(ec == c.NEC - 1))
                    xsq = xpool.tile([P, c.TC], BF16, tag="xsq", name="xsq")
                    nc.vector.tensor_tensor(out=xsq[:], in0=xT[:, ec, :],
                                            in1=xT[:, ec, :], op=OP.mult)
                    nc.tensor.matmul(out=sq_ps[:], lhsT=ones_bf[:], rhs=xsq[:],
                                     start=(ec == 0), stop=(ec == c.NEC - 1))
                stats = xpool.tile([1, 3 * c.TC], F32, tag="stats", name="stats")
                mean = stats[:, 0:c.TC]
                var = stats[:, c.TC:2 * c.TC]
                rstd = stats[:, 2 * c.TC:]
                inv_e = 1.0 / c.E
                nc.scalar.mul(mean, sum_ps[:], inv_e)
                nc.scalar.mul(var, sq_ps[:], inv_e)          # E[x^2]
                m2 = xpool.tile([1, c.TC], F32, tag="m2", name="m2")
                nc.vector.tensor_tensor(out=m2[:], in0=mean, in1=mean,
                                        op=OP.mult)
                nc.vector.tensor_tensor(out=var, in0=var, in1=m2[:],
                                        op=OP.subtract)
                nc.vector.tensor_scalar_add(out=var, in0=var, scalar1=1e-5)
                nc.scalar.activation(var, var, AF.Sqrt)
                nc.vector.reciprocal(rstd, var)
                mrb = xpool.tile([P, 2, c.TC], F32, tag="mrb", name="mrb")
                nc.gpsimd.partition_broadcast(mrb[:, 0, :], mean)
                nc.gpsimd.partition_broadcast(mrb[:, 1, :], rstd)
                for ec in range(c.NEC):
                    tmp = xpool.tile([P, c.TC], F32, tag="lntmp", name="lntmp")
                    nc.vector.tensor_tensor(
                        out=tmp[:], in0=xT[:, ec, :],
                        in1=mrb[:, 0, :], op=OP.subtract)
                    nc.vector.tensor_tensor(
                        out=tmp[:], in0=tmp[:],
                        in1=mrb[:, 1, :], op=OP.mult)
                    nc.vector.tensor_scalar(
                        out=out_bf[:, ec, :], in0=tmp[:],
                        scalar1=gb[:, ec:ec + 1],
                        scalar2=gb[:, c.NEC + ec:c.NEC + ec + 1],
                        op0=OP.mult, op1=OP.add)

            # ================= layers =================
            for l in range(c.L):
                # ---- LN1 ----
                hT = sb.tile([P, c.NEC, c.TC], BF16, tag="hT", name="hT")
                layernorm(ln1g_t[l], ln1b_t[l], hT)

                # ---- q,k projections (transposed output [feat, t]) ----
                qT = sb.tile([P, c.NHP, c.TC], BF16, tag="qT", name="qT")
                kT = sb.tile([P, c.NHP, c.TC], BF16, tag="kT", name="kT")
                for which, dst in ((0, qT), (1, kT)):
                    col0 = which * c.H * c.HS
                    pss = [psum(name=f"qk{fc}") for fc in range(c.NHP)]
                    for ec in range(c.NEC):
                        wt = wpool.tile([P, c.H * c.HS], BF16, tag="wblk",
                                        name="wt")
                        nc.sync.dma_start(
                            wt[:], wqkv_t[l, ec * P:(ec + 1) * P,
                                          col0:col0 + c.H * c.HS])
                        for fc in range(c.NHP):
                            nc.tensor.matmul(
                                out=pss[fc][:, :c.TC],
                                lhsT=wt[:, fc * P:(fc + 1) * P],
                                rhs=hT[:, ec, :],
                                start=(ec == 0), stop=(ec == c.NEC - 1))
                    for fc in range(c.NHP):
                        nc.vector.tensor_copy(out=dst[:, fc, :],
                                              in_=pss[fc][:, :c.TC])

                # ---- v projection (natural layout, ones col per head) ----
                vown = sb.tile([P, c.NTC, c.vrow], BF16, tag="vown",
                               name="vown")
                for h in range(c.H):
                    nc.vector.memset(
                        vown[:, :, h * c.HP + c.HS:h * c.HP + c.HS + 1], 1.0)
                vw = min(512, c.H * c.HS)
                nvh = (c.H * c.HS) // vw
                hs_per_vh = vw // c.HS
                col0 = 2 * c.H * c.HS
                pss = {(tcb, vh): psum(name=f"v{tcb}_{vh}")
                       for tcb in range(c.NTC) for vh in range(nvh)}
                for ec in range(c.NEC):
                    wt = wpool.tile([P, c.H * c.HS], BF16, tag="wblk", name="wt")
                    nc.sync.dma_start(
                        wt[:], wqkv_t[l, ec * P:(ec + 1) * P,
                                      col0:col0 + c.H * c.HS])
                    for tcb in range(c.NTC):
                        for vh in range(nvh):
                            nc.tensor.matmul(
                                out=pss[(tcb, vh)][:, :vw],
                                lhsT=hT[:, ec, tcb * P:(tcb + 1) * P],
                                rhs=wt[:, vh * vw:(vh + 1) * vw],
                                start=(ec == 0), stop=(ec == c.NEC - 1))
                for tcb in range(c.NTC):
                    for vh in range(nvh):
                        for hh in range(hs_per_vh):
                            h = vh * hs_per_vh + hh
                            nc.vector.tensor_copy(
                                out=vown[:, tcb, h * c.HP:h * c.HP + c.HS],
                                in_=pss[(tcb, vh)][:, hh * c.HS:(hh + 1) * c.HS])

                # ---- pair AllGather of (kT | vown) ----
                agi = dram.tile([c.shard_elems], BF16, tag="agi", name="agi")
                ago = dram.tile([2 * c.shard_elems], BF16, tag="ago", name="ago")
                for fc in range(c.NHP):
                    nc.sync.dma_start(
                        agi[fc * P * c.TC:(fc + 1) * P * c.TC]
                        .rearrange("(p q) -> p q", p=P), kT[:, fc, :])
                for tcb in range(c.NTC):
                    o = c.k_elems + tcb * P * c.vrow
                    nc.sync.dma_start(
                        agi[o:o + P * c.vrow].rearrange("(p q) -> p q", p=P),
                        vown[:, tcb, :])
                nc.gpsimd.collective_compute(
                    "AllGather", OP.bypass, replica_groups=PAIRS,
                    ins=[agi[:]], outs=[ago[:]])
                kf = sb.tile([P, c.NHP, c.T], BF16, tag="kf", name="kf")
                vf = sb.tile([P, c.NKC, c.vrow], BF16, tag="vf", name="vf")
                for fc in range(c.NHP):
                    for r in range(2):
                        o = r * c.shard_elems + fc * P * c.TC
                        nc.sync.dma_start(
                            kf[:, fc, r * c.TC:(r + 1) * c.TC],
                            ago[o:o + P * c.TC].rearrange("(p q) -> p q", p=P))
                for kc in range(c.NKC):
                    r, tcb = divmod(kc, c.NTC)
                    o = r * c.shard_elems + c.k_elems + tcb * P * c.vrow
                    nc.sync.dma_start(
                        vf[:, kc, :],
                        ago[o:o + P * c.vrow].rearrange("(p q) -> p q", p=P))

                # ---- attention ----
                attT = sb.tile([P, c.NHP, c.TC], BF16, tag="attT", name="attT")
                for h in range(c.H):
                    hp, hb = divmod(h, c.HPP)
                    p0 = hb * c.HS
                    att_ps = psum(name="att_ps")
                    for kc in range(c.NKC):
                        s_ps = psum(name="s_ps")
                        nc.tensor.matmul(
                            out=s_ps[:, :c.TC],
                            lhsT=kf[p0:p0 + c.HS, hp, kc * P:(kc + 1) * P],
                            rhs=qT[p0:p0 + c.HS, hp, :],
                            start=True, stop=True)
                        ex = xpool.tile([P, c.TC], BF16, tag="ex", name="ex",
                                        bufs=4)
                        nc.scalar.activation(ex[:], s_ps[:, :c.TC], AF.Exp,
                                             scale=c.scale)
                        nc.vector.tensor_tensor(out=ex[:], in0=ex[:],
                                                in1=mask_sb[:, kc, :],
                                                op=OP.mult)
                        nc.tensor.matmul(
                            out=att_ps[:c.HP, :c.TC],
                            lhsT=vf[:, kc, h * c.HP:(h + 1) * c.HP],
                            rhs=ex[:],
                            start=(kc == 0), stop=(kc == c.NKC - 1))
                    rec = xpool.tile([1, c.TC], F32, tag="rec", name="rec",
                                     bufs=2)
                    nc.vector.reciprocal(rec[:], att_ps[c.HS:c.HP, :c.TC])
                    recb = xpool.tile([c.HS, c.TC], F32, tag="recb",
                                      name="recb", bufs=2)
                    nc.gpsimd.partition_broadcast(recb[:], rec[:])
                    nc.vector.tensor_tensor(
                        out=attT[p0:p0 + c.HS, hp, :],
                        in0=att_ps[:c.HS, :c.TC],
                        in1=recb[:], op=OP.mult)

                # ---- Wo projection + bo + residual ----
                bob = sb.tile([P, c.NEC], F32, tag="bob", name="bob", bufs=2)
                nc.sync.dma_start(bob[:],
                                  bo_t[l].rearrange("(ec p) -> p ec", p=P))
                pss = [psum(name=f"wo{eo}") for eo in range(c.NEC)]
                for ec in range(c.NEC):
                    wt = wpool.tile([P, c.E], BF16, tag="wblk", name="wt")
                    nc.sync.dma_start(wt[:], wo_t[l, ec * P:(ec + 1) * P, :])
                    for eo in range(c.NEC):
                        nc.tensor.matmul(
                            out=pss[eo][:, :c.TC],
                            lhsT=wt[:, eo * P:(eo + 1) * P],
                            rhs=attT[:, ec, :],
                            start=(ec == 0), stop=(ec == c.NEC - 1))
                for eo in range(c.NEC):
                    tmp = xpool.tile([P, c.TC], F32, tag="lntmp", name="rtmp")
                    nc.vector.tensor_scalar_add(out=tmp[:],
                                                in0=pss[eo][:, :c.TC],
                                                scalar1=bob[:, eo:eo + 1])
                    nc.vector.tensor_tensor(out=xT[:, eo, :], in0=xT[:, eo, :],
                                            in1=tmp[:], op=OP.add)

                # ---- LN2 + FFN ----
                h2T = sb.tile([P, c.NEC, c.TC], BF16, tag="hT", name="h2T")
                layernorm(ln2g_t[l], ln2b_t[l], h2T)

                b1b = sb.tile([P, c.NFC], F32, tag="b1b", name="b1b", bufs=2)
                nc.sync.dma_start(b1b[:],
                                  b1_t[l].rearrange("(fc p) -> p fc", p=P))
                uT = sb.tile([P, c.NFC, c.TC], BF16, tag="uT", name="uT")
                nblk = (c.NFC + 7) // 8
                for fb in range(nblk):
                    fcs = list(range(fb * 8, min(fb * 8 + 8, c.NFC)))
                    pss = {fc: psum(name=f"u{fc}") for fc in fcs}
                    for ec in range(c.NEC):
                        wt = wpool.tile([P, len(fcs) * P], BF16, tag="wblk",
                                        name="wt")
                        nc.sync.dma_start(
                            wt[:], w1_t[l, ec * P:(ec + 1) * P,
                                        fcs[0] * P:fcs[0] * P + len(fcs) * P])
                        for j, fc in enumerate(fcs):
                            nc.tensor.matmul(
                                out=pss[fc][:, :c.TC],
                                lhsT=wt[:, j * P:(j + 1) * P],
                                rhs=h2T[:, ec, :],
                                start=(ec == 0), stop=(ec == c.NEC - 1))
                    for fc in fcs:
                        nc.scalar.activation(uT[:, fc, :], pss[fc][:, :c.TC],
                                             AF.Relu, bias=b1b[:, fc:fc + 1],
                                             scale=1.0)

                b2b = sb.tile([P, c.NEC], F32, tag="bob", name="b2b", bufs=2)
                nc.sync.dma_start(b2b[:],
                                  b2_t[l].rearrange("(ec p) -> p ec", p=P))
                pss = [psum(name=f"y{eo}") for eo in range(c.NEC)]
                for kc in range(c.NFC):
                    wt = wpool.tile([P, c.E], BF16, tag="wblk", name="wt")
                    nc.sync.dma_start(wt[:], w2_t[l, kc * P:(kc + 1) * P, :])
                    for eo in range(c.NEC):
                        nc.tensor.matmul(
                            out=pss[eo][:, :c.TC],
                            lhsT=wt[:, eo * P:(eo + 1) * P],
                            rhs=uT[:, kc, :],
                            start=(kc == 0), stop=(kc == c.NFC - 1))
                for eo in range(c.NEC):
                    tmp = xpool.tile([P, c.TC], F32, tag="lntmp", name="ytmp")
                    nc.vector.tensor_scalar_add(out=tmp[:],
                                                in0=pss[eo][:, :c.TC],
                                                scalar1=b2b[:, eo:eo + 1])
                    nc.vector.tensor_tensor(out=xT[:, eo, :], in0=xT[:, eo, :],
                                            in1=tmp[:], op=OP.add)

            # ================= final LN + lm_head =================
            xlnT = sb.tile([P, c.NEC, c.TC], BF16, tag="hT", name="xlnT")
            layernorm(lnfg_t, lnfb_t, xlnT)

            per_blk = max(1, 8 // c.NTC)     # vocab chunks per psum block
            vi = 0
            while vi < len(c.vchunks):
                grp = c.vchunks[vi:vi + per_blk]
                cw = sum(w for _, w in grp)
                pss = {(tcb, j): psum(name=f"lg{tcb}_{j}")
                       for tcb in range(c.NTC) for j in range(len(grp))}
                for ec in range(c.NEC):
                    wt = wpool.tile([P, 512 * per_blk], BF16, tag="wblk",
                                    name="wt")
                    nc.sync.dma_start(
                        wt[:, :cw],
                        wh_t[ec * P:(ec + 1) * P, grp[0][0]:grp[0][0] + cw])
                    for tcb in range(c.NTC):
                        o = 0
                        for j, (v0, wv) in enumerate(grp):
                            nc.tensor.matmul(
                                out=pss[(tcb, j)][:, :wv],
                                lhsT=xlnT[:, ec, tcb * P:(tcb + 1) * P],
                                rhs=wt[:, o:o + wv],
                                start=(ec == 0), stop=(ec == c.NEC - 1))
                            o += wv
                bhbs = {}
                for j, (v0, wv) in enumerate(grp):
                    bhb = xpool.tile([P, 512], F32, tag="bhb", name="bhb",
                                     bufs=per_blk + 2)
                    nc.sync.dma_start(
                        bhb[:, :wv],
                        bh_t[None, v0:v0 + wv].to_broadcast([P, wv]))
                    bhbs[j] = bhb
                for tcb in range(c.NTC):
                    for j, (v0, wv) in enumerate(grp):
                        lg = xpool.tile([P, 512], F32, tag="lg", name="lg",
                                        bufs=4)
                        nc.vector.tensor_tensor(
                            out=lg[:, :wv], in0=pss[(tcb, j)][:, :wv],
                            in1=bhbs[j][:, :wv], op=OP.add)
                        nc.sync.dma_start(
                            out_t[tcb * P:(tcb + 1) * P, v0:v0 + wv],
                            lg[:, :wv])
                vi += per_blk

    nc.compile()
    return nc


# ----------------------------------------------------------------------------
# host side
# ----------------------------------------------------------------------------

def prep_inputs(c: Cfg, inputs):
    """Build the 8 per-core input maps from the full model inputs."""
    bf = ml_dtypes.bfloat16
    f32 = np.float32

    idx = np.asarray(inputs["idx"]).astype(np.int32)
    temb = np.asarray(inputs["tok_emb"], f32).astype(bf)
    pos = np.asarray(inputs["pos_emb"], f32)
    Wq, Wk, Wv = (np.asarray(inputs[k], f32) for k in ("Wq", "Wk", "Wv"))
    EHH = c.H * c.HS
    wqkv = np.ascontiguousarray(np.concatenate(
        [w.transpose(0, 2, 1, 3).reshape(c.L, c.E, EHH)
         for w in (Wq, Wk, Wv)], axis=2).astype(bf))

    shared = {
        "temb": temb, "wqkv": wqkv,
        "wo": np.asarray(inputs["Wo"], f32).astype(bf),
        "w1": np.asarray(inputs["W1"], f32).astype(bf),
        "w2": np.asarray(inputs["W2"], f32).astype(bf),
        "wh": np.asarray(inputs["Wh"], f32).astype(bf),
        "bo": np.asarray(inputs["bo"], f32),
        "ln1g": np.asarray(inputs["ln1_g"], f32),
        "ln1b": np.asarray(inputs["ln1_b"], f32),
        "ln2g": np.asarray(inputs["ln2_g"], f32),
        "ln2b": np.asarray(inputs["ln2_b"], f32),
        "b1": np.asarray(inputs["b1"], f32),
        "b2": np.asarray(inputs["b2"], f32),
        "lnfg": np.asarray(inputs["lnf_g"], f32),
        "lnfb": np.asarray(inputs["lnf_b"], f32),
        "bh": np.asarray(inputs["bh"], f32),
    }

    kg = np.arange(c.T)[:, None]
    in_maps = []
    for core in range(N_CORES):
        b, half = divmod(core, 2)
        t0 = half * c.TC
        qg = t0 + np.arange(c.TC)[None, :]
        m = (kg <= qg).astype(bf).reshape(c.NKC, P, c.TC)
        in_maps.append(dict(
            shared,
            idx=np.ascontiguousarray(idx[b, t0:t0 + c.TC]),
            posT=np.ascontiguousarray(pos[t0:t0 + c.TC].T),
            mask=np.ascontiguousarray(m),
        ))
    return in_maps


_CACHE = {}


def _get_program():
    if "nc" not in _CACHE:
        _CACHE["cfg"] = Cfg()
        _CACHE["nc"] = build_program(_CACHE["cfg"])
    return _CACHE["nc"], _CACHE["cfg"]


def kernel(**inputs) -> np.ndarray:
    nc, c = _get_program()
    in_maps = prep_inputs(c, inputs)
    res = bass_utils.run_bass_kernel_spmd(
        nc, in_maps, core_ids=list(range(N_CORES)))
    out = np.empty((c.B, c.T, c.V), np.float32)
    for core in range(N_CORES):
        b, half = divmod(core, 2)
        out[b, half * c.TC:(half + 1) * c.TC] = res.results[core]["out"]
    return out


# revision 10
# speedup vs baseline: 71.5877x; 71.5877x over previous
"""Trainium2 Bass kernel for a 4-layer decoder transformer (B4 T1024 E1024 H16
hs64 F4096 V32000) on 8 NeuronCores.

Sharding: batch(4) x sequence-half(2). Core c handles batch b=c//2, tokens
[half*512, half*512+512) with half=c%2. The residual stream lives in SBUF
transposed (xT: [E, 512], E on partitions) so every matmul has its
contraction dim on partitions. The only cross-core exchange is a pair
AllGather of (kT | v) per layer; causality is enforced with per-core binary
mask inputs so the 8-core program is uniform (SPMD).

PSUM layout: tag "mm" = 4 rotating single-bank tiles, tag "mm4" = one
4-bank [128, 2048] tile used to batch attention scores so exp/mask run as
[128, 2048] ops (4x fewer ACT/DVE dispatches). All matmul phases use
4-bank accumulation blocks so consecutive blocks double-buffer.
"""

import numpy as np
import ml_dtypes

import concourse.bass as bass
import concourse.bacc as bacc
import concourse.mybir as mybir
import concourse.tile as tile
from concourse import bass_utils
from concourse.masks import make_identity

F32 = mybir.dt.float32
BF16 = mybir.dt.bfloat16
I32 = mybir.dt.int32
AF = mybir.ActivationFunctionType
OP = mybir.AluOpType
P = 128

N_CORES = 8
PAIRS = [[0, 1], [2, 3], [4, 5], [6, 7]]


def _chunks(seq, n):
    seq = list(seq)
    return [seq[i:i + n] for i in range(0, len(seq), n)]


class Cfg:
    def __init__(self, B=4, T=1024, E=1024, H=16, HS=64, L=4, F=4096, V=32000):
        self.B, self.T, self.E, self.H, self.HS = B, T, E, H, HS
        self.L, self.F, self.V = L, F, V
        self.TC = T // 2                    # tokens per core
        self.NEC = E // P                   # E chunks (partition tiles)
        self.NTC = self.TC // P             # local token chunks
        self.NKC = T // P                   # global key chunks
        self.NFC = F // P                   # FFN hidden chunks
        self.HPP = P // HS                  # heads per 128-partition tile
        self.NHP = (H * HS) // P            # head-pair tiles
        self.HP = HS + 1                    # augmented per-head stride in v
        self.scale = 1.0 / (E ** 0.5)
        self.vchunks = []
        v0 = 0
        while v0 < V:
            self.vchunks.append((v0, min(512, V - v0)))
            v0 += 512
        self.k_elems = H * HS * self.TC
        self.vrow = H * self.HP
        self.v_elems = self.TC * self.vrow
        self.shard_elems = self.k_elems + self.v_elems
        # attention kc batching: groups of up to 4 key chunks share one
        # 4-bank psum + one fused exp/mask
        self.kc_groups = _chunks(range(self.NKC), 4)


def build_program(c: Cfg, reps: int = 1, ablate=()):
    nc = bacc.Bacc("TRN2", target_bir_lowering=False, debug=False,
                   num_devices=N_CORES)

    # ---- DRAM I/O ----
    dt_ = nc.dram_tensor
    idx_t = dt_("idx", [c.TC], I32, kind="ExternalInput").ap()
    temb_t = dt_("temb", [c.V, c.E], BF16, kind="ExternalInput").ap()
    posT_t = dt_("posT", [c.E, c.TC], F32, kind="ExternalInput").ap()
    wqkv_t = dt_("wqkv", [c.L, c.E, 3 * c.H * c.HS], BF16,
                 kind="ExternalInput").ap()
    wo_t = dt_("wo", [c.L, c.E, c.E], BF16, kind="ExternalInput").ap()
    bo_t = dt_("bo", [c.L, c.E], F32, kind="ExternalInput").ap()
    ln1g_t = dt_("ln1g", [c.L, c.E], F32, kind="ExternalInput").ap()
    ln1b_t = dt_("ln1b", [c.L, c.E], F32, kind="ExternalInput").ap()
    ln2g_t = dt_("ln2g", [c.L, c.E], F32, kind="ExternalInput").ap()
    ln2b_t = dt_("ln2b", [c.L, c.E], F32, kind="ExternalInput").ap()
    w1_t = dt_("w1", [c.L, c.E, c.F], BF16, kind="ExternalInput").ap()
    b1_t = dt_("b1", [c.L, c.F], F32, kind="ExternalInput").ap()
    w2_t = dt_("w2", [c.L, c.F, c.E], BF16, kind="ExternalInput").ap()
    b2_t = dt_("b2", [c.L, c.E], F32, kind="ExternalInput").ap()
    lnfg_t = dt_("lnfg", [c.E], F32, kind="ExternalInput").ap()
    lnfb_t = dt_("lnfb", [c.E], F32, kind="ExternalInput").ap()
    wh_t = dt_("wh", [c.E, c.V], BF16, kind="ExternalInput").ap()
    bh_t = dt_("bh", [c.V], F32, kind="ExternalInput").ap()
    mask_t = dt_("mask", [c.NKC, P, c.TC], BF16, kind="ExternalInput").ap()
    out_t = dt_("out", [c.TC, c.V], F32, kind="ExternalOutput").ap()

    with tile.TileContext(nc) as tc:
        with tc.tile_pool(name="sb", bufs=1) as sb, \
             tc.tile_pool(name="wpool", bufs=6) as wpool, \
             tc.tile_pool(name="xpool", bufs=2) as xpool, \
             tc.tile_pool(name="ps", bufs=4, space="PSUM") as ps, \
             tc.tile_pool(name="dram", bufs=2, space="DRAM") as dram:

            def psum(shape=None, dtype=F32, name="mm"):
                return ps.tile(shape or [P, 512], dtype, tag="mm", name=name)

            def psum4(name="mm4"):
                return ps.tile([P, 4 * 512], F32, tag="mm4", name=name,
                               bufs=1)

            # ---- constants ----
            ones_bf = sb.tile([P, 1], BF16, tag="ones_bf", name="ones_bf")
            nc.vector.memset(ones_bf[:], 1.0)
            ident = sb.tile([P, P], BF16, tag="ident", name="ident")
            make_identity(nc, ident[:])
            mask_sb = sb.tile([P, c.NKC, c.TC], BF16, tag="mask",
                              name="mask_sb")
            for kc in range(c.NKC):
                nc.sync.dma_start(mask_sb[:, kc, :], mask_t[kc])
            idx_sb = sb.tile([P, c.NTC], I32, tag="idx", name="idx_sb")
            nc.sync.dma_start(idx_sb[:], idx_t.rearrange("(tc p) -> p tc", p=P))

            for _rep in range(reps):
                # ---- residual stream xT[e, t] (f32), seeded with pos^T ----
                xT = sb.tile([P, c.NEC, c.TC], F32, tag="xT", name="xT")
                for ec in range(c.NEC):
                    nc.sync.dma_start(xT[:, ec, :],
                                      posT_t[ec * P:(ec + 1) * P, :])

                # ---- embedding gather + transpose ----
                for tcb in range(c.NTC):
                    emb = xpool.tile([P, c.E], BF16, tag="emb", name="emb")
                    if "gather" in ablate:
                        nc.sync.dma_start(emb[:],
                                          temb_t[tcb * P:(tcb + 1) * P, :])
                    else:
                        nc.gpsimd.indirect_dma_start(
                            out=emb[:], out_offset=None, in_=temb_t,
                            in_offset=bass.IndirectOffsetOnAxis(
                                ap=idx_sb[:, tcb:tcb + 1], axis=0))
                    for ec in range(c.NEC):
                        tps = psum([P, P], BF16, name="tps")
                        nc.tensor.transpose(
                            out=tps[:], in_=emb[:, ec * P:(ec + 1) * P],
                            identity=ident[:])
                        tpf = xpool.tile([P, P], F32, tag="tpf", name="tpf")
                        nc.vector.tensor_copy(out=tpf[:], in_=tps[:])
                        sl = xT[:, ec, tcb * P:(tcb + 1) * P]
                        nc.vector.tensor_tensor(out=sl, in0=sl, in1=tpf[:],
                                                op=OP.add)

                # ---- layernorm: xT -> out_bf (bf16 [P, NEC, TC]) ----
                def layernorm(xT, g_dram, b_dram, out_bf):
                    gb = sb.tile([P, 2 * c.NEC], F32, tag="gains", name="gb",
                                 bufs=2)
                    nc.sync.dma_start(
                        gb[:, 0:c.NEC], g_dram.rearrange("(ec p) -> p ec", p=P))
                    nc.sync.dma_start(
                        gb[:, c.NEC:], b_dram.rearrange("(ec p) -> p ec", p=P))
                    sum_ps = psum([1, c.TC], name="ln_sum")
                    sq_ps = psum([1, c.TC], name="ln_sq")
                    for ec in range(c.NEC):
                        xbf = xpool.tile([P, c.TC], BF16, tag="xbf", name="xbf")
                        nc.vector.tensor_copy(out=xbf[:], in_=xT[:, ec, :])
                        nc.tensor.matmul(out=sum_ps[:], lhsT=ones_bf[:],
                                         rhs=xbf[:], start=(ec == 0),
                                         stop=(ec == c.NEC - 1))
                        xsq = xpool.tile([P, c.TC], BF16, tag="xsq", name="xsq")
                        nc.vector.tensor_tensor(out=xsq[:], in0=xT[:, ec, :],
                                                in1=xT[:, ec, :], op=OP.mult)
                        nc.tensor.matmul(out=sq_ps[:], lhsT=ones_bf[:],
                                         rhs=xsq[:], start=(ec == 0),
                                         stop=(ec == c.NEC - 1))
                    stats = xpool.tile([1, 3 * c.TC], F32, tag="stats",
                                       name="stats")
                    mean = stats[:, 0:c.TC]
                    var = stats[:, c.TC:2 * c.TC]
                    rstd = stats[:, 2 * c.TC:]
                    inv_e = 1.0 / c.E
                    nc.scalar.mul(mean, sum_ps[:], inv_e)
                    nc.scalar.mul(var, sq_ps[:], inv_e)
                    m2 = xpool.tile([1, c.TC], F32, tag="m2", name="m2")
                    nc.vector.tensor_tensor(out=m2[:], in0=mean, in1=mean,
                                            op=OP.mult)
                    nc.vector.tensor_tensor(out=var, in0=var, in1=m2[:],
                                            op=OP.subtract)
                    nc.vector.tensor_scalar_add(out=var, in0=var, scalar1=1e-5)
                    nc.scalar.activation(var, var, AF.Sqrt)
                    nc.vector.reciprocal(rstd, var)
                    mrb = xpool.tile([P, 2, c.TC], F32, tag="mrb", name="mrb")
                    nc.gpsimd.partition_broadcast(mrb[:, 0, :], mean)
                    nc.gpsimd.partition_broadcast(mrb[:, 1, :], rstd)
                    for ec in range(c.NEC):
                        tmp = xpool.tile([P, c.TC], F32, tag="lntmp",
                                         name="lntmp")
                        nc.vector.tensor_tensor(out=tmp[:], in0=xT[:, ec, :],
                                                in1=mrb[:, 0, :],
                                                op=OP.subtract)
                        nc.vector.tensor_tensor(out=tmp[:], in0=tmp[:],
                                                in1=mrb[:, 1, :], op=OP.mult)
                        nc.vector.tensor_scalar(
                            out=out_bf[:, ec, :], in0=tmp[:],
                            scalar1=gb[:, ec:ec + 1],
                            scalar2=gb[:, c.NEC + ec:c.NEC + ec + 1],
                            op0=OP.mult, op1=OP.add)

                # ============ layers ============
                for l in range(c.L):
                    hT = sb.tile([P, c.NEC, c.TC], BF16, tag="hT", name="hT")
                    layernorm(xT, ln1g_t[l], ln1b_t[l], hT)

                    # ---- q,k projections (transposed [feat, t]) ----
                    qT = sb.tile([P, c.NHP, c.TC], BF16, tag="qT", name="qT")
                    kT = sb.tile([P, c.NHP, c.TC], BF16, tag="kT", name="kT")
                    qk_jobs = () if "qkv" in ablate else ((0, qT), (1, kT))
                    if "qkv" in ablate:
                        nc.vector.memset(qT[:], 0.0078125)
                        nc.vector.memset(kT[:], 0.0078125)
                    for which, dst in qk_jobs:
                        col0 = which * c.H * c.HS
                        for fcs in _chunks(range(c.NHP), 4):
                            pss = {fc: psum(name=f"qk{fc}") for fc in fcs}
                            for ec in range(c.NEC):
                                wt = wpool.tile([P, len(fcs) * P], BF16,
                                                tag="wblk", name="wt")
                                nc.sync.dma_start(
                                    wt[:],
                                    wqkv_t[l, ec * P:(ec + 1) * P,
                                           col0 + fcs[0] * P:
                                           col0 + fcs[0] * P + len(fcs) * P])
                                for j, fc in enumerate(fcs):
                                    nc.tensor.matmul(
                                        out=pss[fc][:, :c.TC],
                                        lhsT=wt[:, j * P:(j + 1) * P],
                                        rhs=hT[:, ec, :],
                                        start=(ec == 0),
                                        stop=(ec == c.NEC - 1))
                            for fc in fcs:
                                nc.vector.tensor_copy(out=dst[:, fc, :],
                                                      in_=pss[fc][:, :c.TC])

                    # ---- v projection (natural, ones col per head) ----
                    vown = sb.tile([P, c.NTC, c.vrow], BF16, tag="vown",
                                   name="vown")
                    for h in range(c.H):
                        nc.vector.memset(
                            vown[:, :, h * c.HP + c.HS:h * c.HP + c.HS + 1],
                            1.0)
                    vw = min(512, c.H * c.HS)
                    nvh = 0 if "qkv" in ablate else (c.H * c.HS) // vw
                    hs_per_vh = vw // c.HS
                    col0 = 2 * c.H * c.HS
                    vjobs = [(tcb, vh) for tcb in range(c.NTC)
                             for vh in range(nvh)]
                    for grp in _chunks(vjobs, 4):
                        pss = {j: psum(name=f"v{j[0]}_{j[1]}") for j in grp}
                        for ec in range(c.NEC):
                            wts = {}
                            for vh in sorted({vh for _, vh in grp}):
                                wt = wpool.tile([P, vw], BF16, tag="wblk",
                                                name="wt")
                                nc.sync.dma_start(
                                    wt[:],
                                    wqkv_t[l, ec * P:(ec + 1) * P,
                                           col0 + vh * vw:
                                           col0 + (vh + 1) * vw])
                                wts[vh] = wt
                            for tcb, vh in grp:
                                nc.tensor.matmul(
                                    out=pss[(tcb, vh)][:, :vw],
                                    lhsT=hT[:, ec, tcb * P:(tcb + 1) * P],
                                    rhs=wts[vh][:],
                                    start=(ec == 0), stop=(ec == c.NEC - 1))
                        for tcb, vh in grp:
                            for hh in range(hs_per_vh):
                                h = vh * hs_per_vh + hh
                                nc.vector.tensor_copy(
                                    out=vown[:, tcb,
                                             h * c.HP:h * c.HP + c.HS],
                                    in_=pss[(tcb, vh)][:,
                                                       hh * c.HS:
                                                       (hh + 1) * c.HS])

                    # ---- pair AllGather of (kT | vown) ----
                    agi = dram.tile([c.shard_elems], BF16, tag="agi",
                                    name="agi")
                    ago = dram.tile([2 * c.shard_elems], BF16, tag="ago",
                                    name="ago")
                    for fc in range(c.NHP):
                        nc.sync.dma_start(
                            agi[fc * P * c.TC:(fc + 1) * P * c.TC]
                            .rearrange("(p q) -> p q", p=P), kT[:, fc, :])
                    for tcb in range(c.NTC):
                        o = c.k_elems + tcb * P * c.vrow
                        nc.sync.dma_start(
                            agi[o:o + P * c.vrow]
                            .rearrange("(p q) -> p q", p=P), vown[:, tcb, :])
                    if "ag" in ablate:
                        nc.sync.dma_start(ago[0:c.shard_elems], agi[:])
                        nc.sync.dma_start(ago[c.shard_elems:], agi[:])
                    else:
                        nc.gpsimd.collective_compute(
                            "AllGather", OP.bypass, replica_groups=PAIRS,
                            ins=[agi[:]], outs=[ago[:]])
                    kf = sb.tile([P, c.NHP, c.T], BF16, tag="kf", name="kf")
                    vf = sb.tile([P, c.NKC, c.vrow], BF16, tag="vf", name="vf")
                    for fc in range(c.NHP):
                        for r in range(2):
                            o = r * c.shard_elems + fc * P * c.TC
                            nc.sync.dma_start(
                                kf[:, fc, r * c.TC:(r + 1) * c.TC],
                                ago[o:o + P * c.TC]
                                .rearrange("(p q) -> p q", p=P))
                    for kc in range(c.NKC):
                        r, tcb = divmod(kc, c.NTC)
                        o = r * c.shard_elems + c.k_elems + tcb * P * c.vrow
                        nc.sync.dma_start(
                            vf[:, kc, :],
                            ago[o:o + P * c.vrow]
                            .rearrange("(p q) -> p q", p=P))

                    # ---- attention ----
                    attT = sb.tile([P, c.NHP, c.TC], BF16, tag="attT",
                                   name="attT")
                    if "attn" in ablate:
                        nc.vector.memset(attT[:], 0.00390625)
                    for h in range(0 if "attn" in ablate else c.H):
                        hp, hb = divmod(h, c.HPP)
                        p0 = hb * c.HS
                        att_ps = psum(name="att_ps")
                        for kcs in c.kc_groups:
                            s4 = psum4(name="s4")
                            for j, kc in enumerate(kcs):
                                nc.tensor.matmul(
                                    out=s4[:, j * c.TC:(j + 1) * c.TC],
                                    lhsT=kf[p0:p0 + c.HS, hp,
                                            kc * P:(kc + 1) * P],
                                    rhs=qT[p0:p0 + c.HS, hp, :],
                                    start=True, stop=True)
                            nk = len(kcs)
                            ex = xpool.tile([P, 4 * c.TC], BF16, tag="ex",
                                            name="ex", bufs=3)
                            if "exp" in ablate:
                                nc.vector.tensor_copy(
                                    out=ex[:, :nk * c.TC],
                                    in_=s4[:, :nk * c.TC])
                            else:
                                nc.scalar.activation(
                                    ex[:, :nk * c.TC], s4[:, :nk * c.TC],
                                    AF.Exp, scale=c.scale)
                                nc.vector.tensor_tensor(
                                    out=ex[:, :nk * c.TC],
                                    in0=ex[:, :nk * c.TC],
                                    in1=mask_sb[:, kcs[0]:kcs[0] + nk, :],
                                    op=OP.mult)
                            for j, kc in enumerate(kcs):
                                if "av" in ablate:
                                    if kc == 0:
                                        nc.vector.memset(
                                            att_ps[:c.HP, :c.TC], 0.0078125)
                                    continue
                                nc.tensor.matmul(
                                    out=att_ps[:c.HP, :c.TC],
                                    lhsT=vf[:, kc, h * c.HP:(h + 1) * c.HP],
                                    rhs=ex[:, j * c.TC:(j + 1) * c.TC],
                                    start=(kc == 0), stop=(kc == c.NKC - 1))
                        rec = xpool.tile([1, c.TC], F32, tag="rec",
                                         name="rec", bufs=2)
                        nc.vector.reciprocal(rec[:], att_ps[c.HS:c.HP, :c.TC])
                        recb = xpool.tile([c.HS, c.TC], F32, tag="recb",
                                          name="recb", bufs=2)
                        nc.gpsimd.partition_broadcast(recb[:], rec[:])
                        nc.vector.tensor_tensor(
                            out=attT[p0:p0 + c.HS, hp, :],
                            in0=att_ps[:c.HS, :c.TC],
                            in1=recb[:], op=OP.mult)

                    # ---- Wo projection + bo + residual ----
                    bob = sb.tile([P, c.NEC], F32, tag="bob", name="bob",
                                  bufs=2)
                    nc.sync.dma_start(
                        bob[:], bo_t[l].rearrange("(ec p) -> p ec", p=P))
                    for eos in _chunks(range(c.NEC), 4):
                        pss = {eo: psum(name=f"wo{eo}") for eo in eos}
                        for ec in range(c.NEC):
                            wt = wpool.tile([P, len(eos) * P], BF16,
                                            tag="wblk", name="wt")
                            nc.sync.dma_start(
                                wt[:], wo_t[l, ec * P:(ec + 1) * P,
                                            eos[0] * P:
                                            eos[0] * P + len(eos) * P])
                            for j, eo in enumerate(eos):
                                nc.tensor.matmul(
                                    out=pss[eo][:, :c.TC],
                                    lhsT=wt[:, j * P:(j + 1) * P],
                                    rhs=attT[:, ec, :],
                                    start=(ec == 0), stop=(ec == c.NEC - 1))
                        for eo in eos:
                            tmp = xpool.tile([P, c.TC], F32, tag="lntmp",
                                             name="rtmp")
                            nc.vector.tensor_scalar_add(
                                out=tmp[:], in0=pss[eo][:, :c.TC],
                                scalar1=bob[:, eo:eo + 1])
                            nc.vector.tensor_tensor(
                                out=xT[:, eo, :], in0=xT[:, eo, :],
                                in1=tmp[:], op=OP.add)

                    # ---- LN2 + FFN ----
                    h2T = sb.tile([P, c.NEC, c.TC], BF16, tag="hT", name="h2T")
                    layernorm(xT, ln2g_t[l], ln2b_t[l], h2T)

                    b1b = sb.tile([P, c.NFC], F32, tag="b1b", name="b1b",
                                  bufs=2)
                    nc.sync.dma_start(
                        b1b[:], b1_t[l].rearrange("(fc p) -> p fc", p=P))
                    uT = sb.tile([P, c.NFC, c.TC], BF16, tag="uT", name="uT")
                    if "ffn" in ablate:
                        nc.vector.memset(uT[:], 0.0078125)
                    for fcs in ([] if "ffn" in ablate
                                else _chunks(range(c.NFC), 4)):
                        pss = {fc: psum(name=f"u{fc}") for fc in fcs}
                        for ec in range(c.NEC):
                            wt = wpool.tile([P, len(fcs) * P], BF16,
                                            tag="wblk", name="wt")
                            nc.sync.dma_start(
                                wt[:], w1_t[l, ec * P:(ec + 1) * P,
                                            fcs[0] * P:
                                            fcs[0] * P + len(fcs) * P])
                            for j, fc in enumerate(fcs):
                                nc.tensor.matmul(
                                    out=pss[fc][:, :c.TC],
                                    lhsT=wt[:, j * P:(j + 1) * P],
                                    rhs=h2T[:, ec, :],
                                    start=(ec == 0), stop=(ec == c.NEC - 1))
                        for fc in fcs:
                            nc.scalar.activation(
                                uT[:, fc, :], pss[fc][:, :c.TC], AF.Relu,
                                bias=b1b[:, fc:fc + 1], scale=1.0)

                    b2b = sb.tile([P, c.NEC], F32, tag="bob", name="b2b",
                                  bufs=2)
                    nc.sync.dma_start(
                        b2b[:], b2_t[l].rearrange("(ec p) -> p ec", p=P))
                    for eos in ([] if "ffn" in ablate
                                else _chunks(range(c.NEC), 4)):
                        pss = {eo: psum(name=f"y{eo}") for eo in eos}
                        for kc in range(c.NFC):
                            wt = wpool.tile([P, len(eos) * P], BF16,
                                            tag="wblk", name="wt")
                            nc.sync.dma_start(
                                wt[:], w2_t[l, kc * P:(kc + 1) * P,
                                            eos[0] * P:
                                            eos[0] * P + len(eos) * P])
                            for j, eo in enumerate(eos):
                                nc.tensor.matmul(
                                    out=pss[eo][:, :c.TC],
                                    lhsT=wt[:, j * P:(j + 1) * P],
                                    rhs=uT[:, kc, :],
                                    start=(kc == 0), stop=(kc == c.NFC - 1))
                        for eo in eos:
                            tmp = xpool.tile([P, c.TC], F32, tag="lntmp",
                                             name="ytmp")
                            nc.vector.tensor_scalar_add(
                                out=tmp[:], in0=pss[eo][:, :c.TC],
                                scalar1=b2b[:, eo:eo + 1])
                            nc.vector.tensor_tensor(
                                out=xT[:, eo, :], in0=xT[:, eo, :],
                                in1=tmp[:], op=OP.add)

                # ============ final LN + lm_head ============
                xlnT = sb.tile([P, c.NEC, c.TC], BF16, tag="hT", name="xlnT")
                layernorm(xT, lnfg_t, lnfb_t, xlnT)

                vcs = [] if "lmhead" in ablate else c.vchunks
                for v0, wv in vcs:
                    pss = {tcb: psum(name=f"lg{tcb}") for tcb in range(c.NTC)}
                    for ec in range(c.NEC):
                        wt = wpool.tile([P, 512], BF16, tag="wblk", name="wt")
                        nc.sync.dma_start(
                            wt[:, :wv], wh_t[ec * P:(ec + 1) * P, v0:v0 + wv])
                        for tcb in range(c.NTC):
                            nc.tensor.matmul(
                                out=pss[tcb][:, :wv],
                                lhsT=xlnT[:, ec, tcb * P:(tcb + 1) * P],
                                rhs=wt[:, :wv],
                                start=(ec == 0), stop=(ec == c.NEC - 1))
                    bhb = xpool.tile([P, 512], F32, tag="bhb", name="bhb",
                                     bufs=3)
                    nc.sync.dma_start(
                        bhb[:, :wv],
                        bh_t[None, v0:v0 + wv].to_broadcast([P, wv]))
                    for tcb in range(c.NTC):
                        lg = xpool.tile([P, 512], F32, tag="lg", name="lg",
                                        bufs=4)
                        nc.vector.tensor_tensor(
                            out=lg[:, :wv], in0=pss[tcb][:, :wv],
                            in1=bhb[:, :wv], op=OP.add)
                        nc.sync.dma_start(
                            out_t[tcb * P:(tcb + 1) * P, v0:v0 + wv],
                            lg[:, :wv])

    nc.compile()
    return nc


# ----------------------------------------------------------------------------
# host side
# ----------------------------------------------------------------------------

def prep_inputs(c: Cfg, inputs):
    """Build the 8 per-core input maps from the full model inputs."""
    bf = ml_dtypes.bfloat16
    f32 = np.float32

    idx = np.asarray(inputs["idx"]).astype(np.int32)
    temb = np.asarray(inputs["tok_emb"], f32).astype(bf)
    pos = np.asarray(inputs["pos_emb"], f32)
    Wq, Wk, Wv = (np.asarray(inputs[k], f32) for k in ("Wq", "Wk", "Wv"))
    EHH = c.H * c.HS
    wqkv = np.ascontiguousarray(np.concatenate(
        [w.transpose(0, 2, 1, 3).reshape(c.L, c.E, EHH)
         for w in (Wq, Wk, Wv)], axis=2).astype(bf))

    shared = {
        "temb": temb, "wqkv": wqkv,
        "wo": np.asarray(inputs["Wo"], f32).astype(bf),
        "w1": np.asarray(inputs["W1"], f32).astype(bf),
        "w2": np.asarray(inputs["W2"], f32).astype(bf),
        "wh": np.asarray(inputs["Wh"], f32).astype(bf),
        "bo": np.asarray(inputs["bo"], f32),
        "ln1g": np.asarray(inputs["ln1_g"], f32),
        "ln1b": np.asarray(inputs["ln1_b"], f32),
        "ln2g": np.asarray(inputs["ln2_g"], f32),
        "ln2b": np.asarray(inputs["ln2_b"], f32),
        "b1": np.asarray(inputs["b1"], f32),
        "b2": np.asarray(inputs["b2"], f32),
        "lnfg": np.asarray(inputs["lnf_g"], f32),
        "lnfb": np.asarray(inputs["lnf_b"], f32),
        "bh": np.asarray(inputs["bh"], f32),
    }

    kg = np.arange(c.T)[:, None]
    in_maps = []
    for core in range(N_CORES):
        b, half = divmod(core, 2)
        t0 = half * c.TC
        qg = t0 + np.arange(c.TC)[None, :]
        m = (kg <= qg).astype(bf).reshape(c.NKC, P, c.TC)
        in_maps.append(dict(
            shared,
            idx=np.ascontiguousarray(idx[b, t0:t0 + c.TC]),
            posT=np.ascontiguousarray(pos[t0:t0 + c.TC].T),
            mask=np.ascontiguousarray(m),
        ))
    return in_maps


_CACHE = {}


def _get_program():
    if "nc" not in _CACHE:
        _CACHE["cfg"] = Cfg()
        _CACHE["nc"] = build_program(_CACHE["cfg"])
    return _CACHE["nc"], _CACHE["cfg"]


def kernel(**inputs) -> np.ndarray:
    nc, c = _get_program()
    in_maps = prep_inputs(c, inputs)
    res = bass_utils.run_bass_kernel_spmd(
        nc, in_maps, core_ids=list(range(N_CORES)))
    out = np.empty((c.B, c.T, c.V), np.float32)
    for core in range(N_CORES):
        b, half = divmod(core, 2)
        out[b, half * c.TC:(half + 1) * c.TC] = res.results[core]["out"]
    return out
